# revision 9
# baseline (speedup 1.0000x reference)
"""GroupedQueryAttention Trainium2 Bass kernel (8-core SPMD, no collectives).

Module: B=2, S=2048, D=1024, H=16 q-heads, KVH=4 kv-heads, dk=64.
  Q = query @ Wq + bq ; K = key @ Wk + bk ; V = value @ Wv + bv
  out = softmax(Q K^T / 8) V  (GQA groups of 4 q-heads per kv-head)
  output = ctx @ Wo + bo ;  returns (output, K, V)

Sharding: core c handles (batch b=c//4, query-row quarter qc=c%4, 512 rows).
K/V are computed fully per core (inputs batch-replicated), with key/value
rows ROTATED per core so each core's K/V output quarter is the static
slice [0:512] (keeps one SPMD program, softmax is permutation-invariant).

On-chip layouts: activations feature-major (PE transposes), scores computed
transposed [Sk, q] so the probability matrix feeds the ctx matmul with no
further transposes. Softmax denominator comes from a ones-column appended
to V; normalization is applied to ctx^T via a PE outer-product broadcast.
Wq columns / Wo rows (and bq) are host-permuted so that each PSUM
projection tile holds one head at partitions 0:64 and one at 64:128 with
the kv-group parity matching K^T's partition placement (matmul requires
lhsT/rhs to share their base partition).
"""

import numpy as np

import concourse.bacc as bacc
import concourse.bass as bass
import concourse.tile as tile
from concourse import mybir

B = 2
S = 2048
D = 1024
H = 16
KVH = 4
DK = 64
P = 128
QR = S // 4  # query rows per core = 512
NCORES = 8
F32 = mybir.dt.float32

# head -> (partition offset, slot) map; HA[t]/HB[t] = heads of slot t at
# partitions 0:64 / 64:128. Parity of kv-group (h//4) selects the half so
# that Q^T head slices align with K^T's kv-head partition placement.
HA = [0, 1, 2, 3, 8, 9, 10, 11]
HB = [4, 5, 6, 7, 12, 13, 14, 15]


def _slot(h):
    return (h % 4) + 4 * (h // 8)


def _poff(h):
    return 64 * ((h // 4) % 2)


def build_nc(compute_dt=F32, phases="ABCkvqr"):
    nc = bacc.Bacc(None, target_bir_lowering=False, debug=False)

    q_sl = nc.dram_tensor("q_sl", [QR, D], F32, kind="ExternalInput")
    key_r = nc.dram_tensor("key_r", [S, D], F32, kind="ExternalInput")
    val_r = nc.dram_tensor("val_r", [S, D], F32, kind="ExternalInput")
    wq_d = nc.dram_tensor("wq_d", [D, D], F32, kind="ExternalInput")
    wk_d = nc.dram_tensor("wk_d", [D, KVH * DK], F32, kind="ExternalInput")
    wv_d = nc.dram_tensor("wv_d", [D, KVH * DK], F32, kind="ExternalInput")
    wo_d = nc.dram_tensor("wo_d", [D, D], F32, kind="ExternalInput")
    bqt_d = nc.dram_tensor("bqt_d", [P, 8], F32, kind="ExternalInput")
    bkt_d = nc.dram_tensor("bkt_d", [P, 2], F32, kind="ExternalInput")
    bva_d = nc.dram_tensor("bva_d", [KVH * 65], F32, kind="ExternalInput")
    bo_d = nc.dram_tensor("bo_d", [D], F32, kind="ExternalInput")
    id128_d = nc.dram_tensor("id128_d", [P, P], F32, kind="ExternalInput")
    id64x2_d = nc.dram_tensor("id64x2_d", [P, 64], F32, kind="ExternalInput")

    out_sl = nc.dram_tensor("out_sl", [QR, D], F32, kind="ExternalOutput")
    k_out = nc.dram_tensor("k_out", [KVH, QR, DK], F32, kind="ExternalOutput")
    v_out = nc.dram_tensor("v_out", [KVH, QR, DK], F32, kind="ExternalOutput")

    with tile.TileContext(nc) as tc:
        with (
            tc.tile_pool(name="const", bufs=1) as const,
            tc.tile_pool(name="bigw", bufs=1) as bigw,
            tc.tile_pool(name="persist", bufs=1) as persist,
            tc.tile_pool(name="small", bufs=3) as small,
        ):
            # ---- constants ----
            wk_sb = const.tile([P, 8, KVH * DK], F32, tag="wk")
            nc.sync.dma_start(out=wk_sb, in_=wk_d[:].rearrange("(t p) n -> p t n", p=P))
            wv_sb = const.tile([P, 8, KVH * DK], F32, tag="wv")
            nc.sync.dma_start(out=wv_sb, in_=wv_d[:].rearrange("(t p) n -> p t n", p=P))
            bqt_sb = const.tile([P, 8], F32, tag="bqt")
            nc.sync.dma_start(out=bqt_sb, in_=bqt_d[:])
            bkt_sb = const.tile([P, 2], F32, tag="bkt")
            nc.sync.dma_start(out=bkt_sb, in_=bkt_d[:])
            bv_bc = const.tile([P, KVH, 65], F32, tag="bvbc")
            nc.sync.dma_start(
                out=bv_bc,
                in_=bass.AP(
                    tensor=bva_d, offset=0, ap=[[0, P], [65, KVH], [1, 65]]
                ),
            )
            bo_bc = const.tile([P, D], F32, tag="bobc")
            nc.sync.dma_start(
                out=bo_bc,
                in_=bass.AP(tensor=bo_d, offset=0, ap=[[0, P], [1, D]]),
            )
            id128 = const.tile([P, P], F32, tag="id128")
            nc.sync.dma_start(out=id128, in_=id128_d[:])
            id64x2 = const.tile([P, 64], F32, tag="id64")
            nc.sync.dma_start(out=id64x2, in_=id64x2_d[:])
            ones_sb = const.tile([1, 64], F32, tag="ones")
            nc.vector.memset(ones_sb, 1.0)

            wq_sb = bigw.tile([P, 8, D], F32, tag="bigw")
            nc.sync.dma_start(out=wq_sb, in_=wq_d[:].rearrange("(t p) n -> p t n", p=P))

            # ---- persistent activations ----
            kt_sb = persist.tile([P, 2, S], compute_dt, tag="kt")
            v_sb = persist.tile([P, 16, KVH, 65], compute_dt, tag="v")
            qt_sb = persist.tile([P, 8, QR], compute_dt, tag="qt")
            ctxt_sb = persist.tile([P, 8, QR], compute_dt, tag="ctxt")
            nc.vector.memset(v_sb[:, :, :, 64:65], 1.0)

            # ================= Phase A: projections =================
            with (
                tc.tile_pool(name="inch", bufs=2) as inch,
                tc.tile_pool(name="tT", bufs=10) as tTp,
                tc.tile_pool(name="psA", bufs=2, space="PSUM") as psA,
            ):

                def load_and_transpose(src_ap, cpy_i=[0]):
                    """Load 512 rows x D, return 8 feature-major [P,512] tiles."""
                    in_t = inch.tile([P, 4, D], F32, tag="inch")
                    nc.sync.dma_start(
                        out=in_t,
                        in_=src_ap.rearrange("(t p) d -> p t d", p=P),
                    )
                    tts = []
                    for d in range(8):
                        ptt = psA.tile([P, 512], F32, tag="ptt")
                        for r in range(4):
                            nc.tensor.transpose(
                                ptt[:, r * P : (r + 1) * P],
                                in_t[:, r, d * P : (d + 1) * P],
                                id128,
                            )
                        tt = tTp.tile([P, 512], compute_dt, tag="tT")
                        eng = nc.vector if cpy_i[0] % 2 == 0 else nc.scalar
                        if eng is nc.vector:
                            nc.vector.tensor_copy(out=tt, in_=ptt)
                        else:
                            nc.scalar.copy(out=tt, in_=ptt)
                        cpy_i[0] += 1
                        tts.append(tt)
                    return tts

                # K^T projection (feature-major K) over 4 chunks of 512 rows
                for c in range(4 if "k" in phases else 0):
                    tts = load_and_transpose(key_r[c * 512 : (c + 1) * 512, :])
                    for t in range(2):
                        pp = psA.tile([P, 512], F32, tag="pp")
                        for d in range(8):
                            nc.tensor.matmul(
                                pp,
                                wk_sb[:, d, t * P : (t + 1) * P],
                                tts[d],
                                start=(d == 0),
                                stop=(d == 7),
                            )
                        nc.vector.tensor_scalar_add(
                            out=kt_sb[:, t, c * 512 : (c + 1) * 512],
                            in0=pp,
                            scalar1=bkt_sb[:, t : t + 1],
                        )

                # V projection (row-major V with ones column)
                for c in range(4 if "v" in phases else 0):
                    tts = load_and_transpose(val_r[c * 512 : (c + 1) * 512, :])
                    for r in range(4):
                        sk = c * 4 + r
                        pp = psA.tile([P, 512], F32, tag="pp")
                        ppv = pp[:, 0 : KVH * DK]
                        for d in range(8):
                            nc.tensor.matmul(
                                ppv,
                                tts[d][:, r * P : (r + 1) * P],
                                wv_sb[:, d, :],
                                start=(d == 0),
                                stop=(d == 7),
                            )
                        nc.vector.tensor_add(
                            out=v_sb[:, sk, :, 0:64],
                            in0=ppv.rearrange("p (k e) -> p k e", k=KVH),
                            in1=bv_bc[:, :, 0:64],
                        )

                # Q^T projection (this core's 512 query rows)
                tts = load_and_transpose(q_sl[:, :]) if "q" in phases else None
                for t in range(8 if "q" in phases else 0):
                    pp = psA.tile([P, 512], F32, tag="pp")
                    for d in range(8):
                        nc.tensor.matmul(
                            pp,
                            wq_sb[:, d, t * P : (t + 1) * P],
                            tts[d],
                            start=(d == 0),
                            stop=(d == 7),
                        )
                    nc.vector.tensor_scalar_add(
                        out=qt_sb[:, t, :], in0=pp, scalar1=bqt_sb[:, t : t + 1]
                    )

                # K output quarter: transpose K^T cols [0:512] back to rows
                # transposing a full [128,128] K^T block yields both kv-heads
                # of that slot side by side (base-64 transposes hang on HW)
                for j in range(4 if "r" in phases else 0):
                    ptt = psA.tile([P, 512], F32, tag="ptt")
                    for t in range(2):
                        nc.tensor.transpose(
                            ptt[:, t * P : (t + 1) * P],
                            kt_sb[:, t, j * P : (j + 1) * P],
                            id128,
                        )
                    kst = small.tile([P, KVH, DK], F32, tag="kst")
                    nc.vector.tensor_copy(
                        out=kst,
                        in_=ptt[:, 0 : KVH * DK].rearrange("p (k e) -> p k e", k=KVH),
                    )
                    nc.sync.dma_start(
                        out=k_out[:, j * P : (j + 1) * P, :].rearrange(
                            "k p e -> p k e"
                        ),
                        in_=kst,
                    )

                # V output quarter straight from v_sb (sk-tiles 0..3)
                if "r" not in phases:
                    return_early = True
                vst = small.tile([P, 4, KVH, DK], F32, tag="vst")
                nc.vector.tensor_copy(out=vst, in_=v_sb[:, 0:4, :, 0:64])
                for kv in range(KVH):
                    nc.sync.dma_start(
                        out=v_out[kv].rearrange("(t p) e -> p t e", p=P),
                        in_=vst[:, :, kv, :],
                    )

            # load Wo into the bigw slot (waits for last Wq read; overlaps B)
            wo_sb = bigw.tile([P, 8, D], compute_dt, tag="bigw")
            nc.sync.dma_start(out=wo_sb, in_=wo_d[:].rearrange("(t p) n -> p t n", p=P))

            # ================= Phase B: attention =================
            with (
                tc.tile_pool(name="psB", bufs=2, space="PSUM") as psB,
                tc.tile_pool(name="expp", bufs=3) as expp,
                tc.tile_pool(name="outp", bufs=3) as outp,
            ):
                for h in range(H if "B" in phases else 0):
                    kv = h // 4
                    po = _poff(h)
                    sl = _slot(h)
                    ctx_ps = psB.tile([P, QR], F32, tag="ctx")
                    for sk2 in range(8):
                        sc = psB.tile([P, 1024], F32, tag="sc")
                        for half in range(2):
                            sk = sk2 * 2 + half
                            nc.tensor.matmul(
                                sc[:, half * 512 : (half + 1) * 512],
                                kt_sb[po : po + 64, kv // 2, sk * P : (sk + 1) * P],
                                qt_sb[po : po + 64, sl, :],
                                start=True,
                                stop=True,
                            )
                        ex = expp.tile([P, 1024], compute_dt, tag="ex")
                        nc.scalar.activation(
                            out=ex,
                            in_=sc,
                            func=mybir.ActivationFunctionType.Exp,
                            scale=0.125,
                        )
                        for half in range(2):
                            sk = sk2 * 2 + half
                            nc.tensor.matmul(
                                ctx_ps[0:65, :],
                                v_sb[:, sk, kv, :],
                                ex[:, half * 512 : (half + 1) * 512],
                                start=(sk == 0),
                                stop=(sk == 15),
                                skip_group_check=True,
                            )
                    rc = small.tile([1, QR], F32, tag="rc")
                    nc.vector.reciprocal(out=rc, in_=ctx_ps[64:65, :])
                    bc = psB.tile([64, QR], F32, tag="bc")
                    nc.tensor.matmul(
                        bc, ones_sb[0:1, :], rc, start=True, stop=True
                    )
                    bc_sb = small.tile([64, QR], compute_dt, tag="bcsb")
                    nc.vector.tensor_copy(out=bc_sb, in_=bc)
                    nc.vector.tensor_mul(
                        out=ctxt_sb[po : po + 64, sl, :],
                        in0=ctx_ps[0:64, :],
                        in1=bc_sb,
                    )

                # ============= Phase C: output projection =============
                for qs in range(4 if "C" in phases else 0):
                    for n in range(2):
                        pout = psB.tile([P, 512], F32, tag="sc")
                        for t in range(8):
                            nc.tensor.matmul(
                                pout,
                                ctxt_sb[:, t, qs * P : (qs + 1) * P],
                                wo_sb[:, t, n * 512 : (n + 1) * 512],
                                start=(t == 0),
                                stop=(t == 7),
                            )
                        ot = outp.tile([P, 512], F32, tag="ot")
                        nc.vector.tensor_add(
                            out=ot, in0=pout, in1=bo_bc[:, n * 512 : (n + 1) * 512]
                        )
                        nc.sync.dma_start(
                            out=out_sl[qs * P : (qs + 1) * P, n * 512 : (n + 1) * 512],
                            in_=ot,
                        )

    nc.compile()
    return nc


_NC_CACHE = {}


def _get_nc():
    if "nc" not in _NC_CACHE:
        _NC_CACHE["nc"] = build_nc()
    return _NC_CACHE["nc"]


def _prep_in_maps(query, key_in, value_in, Wq, bq, Wk, bk, Wv, bv, Wo, bo):
    query = np.ascontiguousarray(query, np.float32)
    key_in = np.ascontiguousarray(key_in, np.float32)
    value_in = np.ascontiguousarray(value_in, np.float32)

    # permute Wq columns / Wo rows (and bq) to the head-slot layout
    wq_p = np.empty_like(np.asarray(Wq, np.float32))
    bq_p = np.empty_like(np.asarray(bq, np.float32))
    wo_p = np.empty_like(np.asarray(Wo, np.float32))
    for t in range(8):
        ha, hb = HA[t], HB[t]
        wq_p[:, t * 128 : t * 128 + 64] = Wq[:, ha * 64 : (ha + 1) * 64]
        wq_p[:, t * 128 + 64 : (t + 1) * 128] = Wq[:, hb * 64 : (hb + 1) * 64]
        bq_p[t * 128 : t * 128 + 64] = bq[ha * 64 : (ha + 1) * 64]
        bq_p[t * 128 + 64 : (t + 1) * 128] = bq[hb * 64 : (hb + 1) * 64]
        wo_p[t * 128 : t * 128 + 64, :] = Wo[ha * 64 : (ha + 1) * 64, :]
        wo_p[t * 128 + 64 : (t + 1) * 128, :] = Wo[hb * 64 : (hb + 1) * 64, :]

    bqt = np.ascontiguousarray(bq_p.reshape(8, 128).T, np.float32)
    bkt = np.ascontiguousarray(np.asarray(bk, np.float32).reshape(2, 128).T)
    bva = np.zeros((KVH, 65), np.float32)
    bva[:, :64] = np.asarray(bv, np.float32).reshape(KVH, 64)
    id128 = np.eye(128, dtype=np.float32)
    id64x2 = np.concatenate([np.eye(64), np.eye(64)]).astype(np.float32)

    shared = {
        "wq_d": wq_p,
        "wk_d": np.ascontiguousarray(Wk, np.float32),
        "wv_d": np.ascontiguousarray(Wv, np.float32),
        "wo_d": np.ascontiguousarray(wo_p, np.float32),
        "bqt_d": bqt,
        "bkt_d": bkt,
        "bva_d": bva.reshape(-1),
        "bo_d": np.ascontiguousarray(bo, np.float32),
        "id128_d": id128,
        "id64x2_d": id64x2,
    }
    in_maps = []
    for c in range(NCORES):
        b, qc = c // 4, c % 4
        rot = np.roll(np.arange(S), -qc * QR)
        in_maps.append(
            dict(
                shared,
                q_sl=np.ascontiguousarray(query[b, qc * QR : (qc + 1) * QR, :]),
                key_r=np.ascontiguousarray(key_in[b][rot]),
                val_r=np.ascontiguousarray(value_in[b][rot]),
            )
        )
    return in_maps


def _assemble(results):
    out = np.empty((B, S, D), np.float32)
    K = np.empty((B, KVH, S, DK), np.float32)
    V = np.empty((B, KVH, S, DK), np.float32)
    for c in range(NCORES):
        b, qc = c // 4, c % 4
        r = results[c]
        out[b, qc * QR : (qc + 1) * QR, :] = r["out_sl"]
        K[b, :, qc * QR : (qc + 1) * QR, :] = r["k_out"]
        V[b, :, qc * QR : (qc + 1) * QR, :] = r["v_out"]
    return out, K, V


def kernel(query, key_in, value_in, Wq, bq, Wk, bk, Wv, bv, Wo, bo, **run_kwargs):
    from concourse.bass_utils import run_bass_kernel_spmd

    nc = _get_nc()
    in_maps = _prep_in_maps(
        query, key_in, value_in, Wq, bq, Wk, bk, Wv, bv, Wo, bo
    )
    res = run_bass_kernel_spmd(nc, in_maps, list(range(NCORES)), **run_kwargs)
    out = _assemble(res.results)
    if run_kwargs:
        return out, res
    return out
